# revision 27
# baseline (speedup 1.0000x reference)
"""MoE layer kernel for 8 Trainium2 NeuronCores.

Strategy (expert-parallel, matching the sharding hint):
  - Host computes the router (logits/softmax/top-2/gates/aux_loss) with
    eager jax on CPU, replicating reference ops bit-exactly so the top-k
    routing decisions match the oracle.
  - Tokens are dispatched to experts on the host (the "all-to-all"):
    core e receives the (padded) set of tokens routed to expert e,
    pre-transposed to [C, cap] so the device kernel never transposes.
  - Each core runs a dense FFN for its expert:
        hT = gelu(w1^T @ xT + b1);  yT = w2^T @ hT + b2
    entirely as [128,128] x [128,TB] matmuls accumulated in PSUM.
  - Host combines: out[tok] += gate * y[tok] and returns (out, aux_loss).
"""

import os
import functools
import numpy as np
import ml_dtypes

P = 128
C = 768          # embed dim
F = 3072         # ffn dim
E = 8            # experts
TOPK = 2
CCH = C // P     # 6 chunks of embed dim
FCH = F // P     # 24 chunks of ffn dim

# matmul dtype: "bf16" | "fp32r" | "fp32"  (fp32r = fp32 data, TF32-like PE mode)
# fp32r: fp32 storage, TF32-like PE mode. Measured on HW: same PE issue
# rate as bf16 at N>=256 but ~15x lower error (2.4e-4 vs 3.6e-3 absmax
# rel vs the fp32 oracle), and fp32r matmuls beat bf16 end-to-end here
# because bf16 pays a serialized per-MM LDWEIGHTS.
MM_MODE = os.environ.get("MOE_MM_DT", "fp32r")
_TRACE = os.environ.get("MOE_TRACE", "0") == "1"
# CoreSim doesn't implement Gelu; set MOE_SIM_ACT=identity for sim-only runs
_ACT_OVERRIDE = os.environ.get("MOE_SIM_ACT", "")
# Enable walrus's LDWEIGHTS optimization (background weight-buffer loads).
# concourse hardcodes --enable-ldw-opt=false; without it every MATMUL
# serializes behind its weight load (+~46ns/MM measured).
LDW_OPT = os.environ.get("MOE_LDW_OPT", "0") == "1"


def _install_ldw_opt_patch():
    import concourse.bass_utils as bu

    if getattr(bu, "_moe_ldw_patch", None) is not None:
        return
    orig = bu.run_command

    def run_command_ldw(argv, **kwargs):
        if LDW_OPT and isinstance(argv, (list, tuple)):
            argv = [
                "--enable-ldw-opt=true" if a == "--enable-ldw-opt=false" else a
                for a in argv
            ]
        return orig(argv, **kwargs)

    bu.run_command = run_command_ldw
    bu._moe_ldw_patch = orig

LAST_RESULTS = {}  # test.py introspects this for exec_time_ns / profile


def _mm_cfg():
    import concourse.mybir as mybir

    if MM_MODE == "bf16":
        return mybir.dt.bfloat16, np.dtype(ml_dtypes.bfloat16), 512
    if MM_MODE == "fp32r":
        return mybir.dt.float32r, np.dtype(np.float32), 512
    if MM_MODE == "fp32":
        return mybir.dt.float32, np.dtype(np.float32), 256
    raise ValueError(MM_MODE)


@functools.lru_cache(maxsize=4)
def _build_nc(cap: int):
    import concourse.bass as bass
    import concourse.mybir as mybir
    import concourse.tile as tile
    from concourse import bacc

    mm_dt, _, tb = _mm_cfg()
    f32 = mybir.dt.float32
    act = mybir.ActivationFunctionType
    assert cap % 256 == 0
    # token blocks: full-size tb blocks plus an optional 256 remainder
    blocks = []
    t0 = 0
    while t0 < cap:
        bsz = tb if cap - t0 >= tb else cap - t0
        blocks.append((t0, bsz))
        t0 += bsz

    if LDW_OPT:
        _install_ldw_opt_patch()

    # Bacc (not raw Bass): its compile pipeline splits multi-sem waits
    # (TRN2 allows only 1 wait per instruction) and moves matmul waits
    # onto ldweights.
    if LDW_OPT:
        # walrus --enable-ldw-opt rejects explicit InstLdweights, which
        # Bacc's wait-moving pass would emit; skip it and let
        # generate_event_semaphores split those waits instead.
        class _MoeBacc(bacc.Bacc):
            def move_matmul_waits_to_ldweights(self):
                return

        bass_cls = _MoeBacc
    else:
        bass_cls = bacc.Bacc
    nc = bass_cls("TRN2", target_bir_lowering=False, debug=False)
    xT = nc.dram_tensor("xT", [C, cap], mm_dt, kind="ExternalInput").ap()
    w1 = nc.dram_tensor("w1", [C, F], mm_dt, kind="ExternalInput").ap()
    b1 = nc.dram_tensor("b1", [F], f32, kind="ExternalInput").ap()
    w2 = nc.dram_tensor("w2", [F, C], mm_dt, kind="ExternalInput").ap()
    b2 = nc.dram_tensor("b2", [C], f32, kind="ExternalInput").ap()
    yT = nc.dram_tensor("yT", [C, cap], f32, kind="ExternalOutput").ap()

    # h (and therefore the second matmul inputs) use the same dtype as mm
    h_dt = mm_dt

    # fp32r (4-byte) doesn't fit both weight matrices in SBUF; stream w2
    # per output-chunk instead of keeping it resident.
    stream_w2 = MM_MODE in ("fp32r", "fp32")
    gelu_fn = act.Identity if _ACT_OVERRIDE == "identity" else act.Gelu

    with tile.TileContext(nc) as tc:
        with (
            tc.tile_pool(name="wpool", bufs=1) as wpool,
            tc.tile_pool(name="w2pool", bufs=3 if stream_w2 else 1) as w2pool,
            tc.tile_pool(name="xpool", bufs=1 if stream_w2 else 2) as xpool,
            tc.tile_pool(name="hpool", bufs=1 if stream_w2 else 2) as hpool,
            tc.tile_pool(name="ypool", bufs=1 if stream_w2 else 2) as ypool,
            tc.tile_pool(name="pspool", bufs=4, space="PSUM") as pspool,
            tc.tile_pool(name="ps2pool", bufs=3, space="PSUM") as ps2pool,
        ):
            # Two HWDGE rings exist (SP + ACT); alternate them so no single
            # ring's ~300GB/s limits weight/activation delivery.
            def ring(i):
                return nc.sync if i % 2 == 0 else nc.scalar

            def load_x(t0, bsz, first=False):
                x_sb = xpool.tile([P, CCH * tb], mm_dt, name="x_sb", tag="x")
                for c in range(CCH):
                    eng = nc.sync if first else ring(c)
                    eng.dma_start(out=x_sb[:, c * bsz:(c + 1) * bsz],
                                  in_=xT[c * P:(c + 1) * P, t0:t0 + bsz])
                return x_sb

            def load_w2(ci):
                # w2s[p, f*128 + c] = w2[f*128 + p, ci*128 + c]
                # starts on the ACT ring so block 0's first chunk is not
                # queued behind the 9MB w1 load on the SP ring
                w2s = w2pool.tile([P, FCH * P], h_dt, name="w2s", tag="w2s")
                ring(ci + 1).dma_start(
                    out=w2s.rearrange("p (f c) -> p f c", f=FCH),
                    in_=w2[:, ci * P:(ci + 1) * P].rearrange("(f p) c -> p f c", p=P))
                return w2s

            # DMA order matters: biases first (the first gelu gates psum
            # recycling), then block-0 activations, then w1 fi-column tiles
            # (matmul 1 consumes them in order at about the rate one HWDGE
            # ring delivers them).
            # name carries the compile config so the XLA/NEFF cache key
            # changes when the walrus flag set changes
            b1_sb = wpool.tile([P, FCH], f32, name=f"b1_sb_ldw{int(LDW_OPT)}")
            nc.sync.dma_start(out=b1_sb[:, :], in_=b1.rearrange("(a p) -> p a", p=P))
            b2_sb = wpool.tile([P, CCH], f32, name="b2_sb")
            nc.sync.dma_start(out=b2_sb[:, :], in_=b2.rearrange("(a p) -> p a", p=P))

            x_first = load_x(*blocks[0], first=True)

            # w1_sb tile fi: [p, c*128 + m] = w1[c*128 + p, fi*128 + m]
            w1_sb = wpool.tile([P, FCH * CCH * P], mm_dt, name="w1_sb")
            for fi in range(FCH):
                nc.sync.dma_start(
                    out=w1_sb[:, fi * CCH * P:(fi + 1) * CCH * P]
                    .rearrange("p (c m) -> p c m", c=CCH),
                    in_=w1[:, fi * P:(fi + 1) * P].rearrange("(c p) m -> p c m", p=P))

            if not stream_w2:
                # resident w2: tile f: [p, f*C + c] = w2[f*128 + p, c]
                w2_sb = wpool.tile([P, FCH * C], h_dt, name="w2_sb")
                for f in range(FCH):
                    nc.sync.dma_start(out=w2_sb[:, f * C:(f + 1) * C],
                                      in_=w2[f * P:(f + 1) * P, :])

            # --- token-block loop -------------------------------------------------
            x_sb = x_first
            for t, (t0, bsz) in enumerate(blocks):
                h_sb = hpool.tile([P, FCH * tb], h_dt, name="h_sb", tag="h")
                for fi in range(FCH):
                    ps = pspool.tile([P, tb], f32, name="ps", tag="ps")
                    for c in range(CCH):
                        nc.tensor.matmul(
                            ps[:, :bsz],
                            w1_sb[:, (fi * CCH + c) * P:(fi * CCH + c + 1) * P],
                            x_sb[:, c * bsz:(c + 1) * bsz],
                            start=(c == 0),
                            stop=(c == CCH - 1),
                        )
                    nc.scalar.activation(h_sb[:, fi * bsz:(fi + 1) * bsz], ps[:, :bsz],
                                         gelu_fn, bias=b1_sb[:, fi:fi + 1], scale=1.0)

                # prefetch next block's activations once matmul 1 is done
                # reading the (single-buffered) x slot
                if t + 1 < len(blocks):
                    x_sb = load_x(*blocks[t + 1])

                y_sb = ypool.tile([P, CCH * tb], f32, name="y_sb", tag="y")
                for ci in range(CCH):
                    w2t = load_w2(ci) if stream_w2 else None
                    ps2 = ps2pool.tile([P, tb], f32, name="ps2", tag="ps2")
                    for fi in range(FCH):
                        lhsT = (w2t[:, fi * P:(fi + 1) * P] if stream_w2
                                else w2_sb[:, fi * C + ci * P: fi * C + ci * P + P])
                        nc.tensor.matmul(
                            ps2[:, :bsz],
                            lhsT,
                            h_sb[:, fi * bsz:(fi + 1) * bsz],
                            start=(fi == 0),
                            stop=(fi == FCH - 1),
                        )
                    nc.scalar.activation(y_sb[:, ci * bsz:(ci + 1) * bsz], ps2[:, :bsz],
                                         act.Identity, bias=b2_sb[:, ci:ci + 1], scale=1.0)
                    nc.sync.dma_start(out=yT[ci * P:(ci + 1) * P, t0:t0 + bsz],
                                      in_=y_sb[:, ci * bsz:(ci + 1) * bsz])
    # Bacc's compile pipeline (register alloc, wait splitting) runs in
    # finalize(); the axon/PJRT execute path does not call it for us.
    nc.finalize()
    return nc


def _route_host(x, router_w):
    """Replicates the reference router bit-exactly with eager jax on CPU."""
    import jax
    import jax.numpy as jnp

    cpu = jax.devices("cpu")[0]
    with jax.default_device(cpu):
        flat = jnp.asarray(np.asarray(x, np.float32)).reshape(-1, C)
        rw = jnp.asarray(np.asarray(router_w, np.float32))
        logits = flat @ rw.T
        probs = jax.nn.softmax(logits.astype(jnp.float32), axis=-1)
        top_w, top_i = jax.lax.top_k(probs, TOPK)
        top_w = (top_w / top_w.sum(-1, keepdims=True)).astype(flat.dtype)
        one_hot = jax.nn.one_hot(top_i, E, dtype=jnp.float32).sum(axis=1)
        f = one_hot.mean(axis=0)
        pmean = probs.mean(axis=0)
        aux_loss = E * (f * pmean).sum()
        return (np.asarray(top_w), np.asarray(top_i), np.asarray(aux_loss))


def kernel(x, router_w, w1, b1, w2, b2):
    from concourse.bass_utils import run_bass_kernel_spmd

    x = np.asarray(x)
    B, T, _ = x.shape
    N = B * T
    flat = np.ascontiguousarray(x.reshape(N, C).astype(np.float32, copy=False))

    top_w, top_i, aux_loss = _route_host(x, router_w)

    # token index + gate list per expert
    idxs, gates = [], []
    for e in range(E):
        sel0 = np.nonzero(top_i[:, 0] == e)[0]
        sel1 = np.nonzero(top_i[:, 1] == e)[0]
        idxs.append(np.concatenate([sel0, sel1]))
        gates.append(np.concatenate([top_w[sel0, 0], top_w[sel1, 1]]).astype(np.float32))

    _, np_dt, tb = _mm_cfg()
    max_n = max(len(i) for i in idxs)
    cap = max(256, ((max_n + 255) // 256) * 256)

    nc = _build_nc(cap)

    in_maps = []
    for e in range(E):
        xTe = np.zeros((C, cap), np_dt)
        ne = len(idxs[e])
        xTe[:, :ne] = flat[idxs[e]].T.astype(np_dt)
        in_maps.append({
            "xT": xTe,
            "w1": np.ascontiguousarray(w1[e]).astype(np_dt),
            "b1": np.ascontiguousarray(b1[e]).astype(np.float32),
            "w2": np.ascontiguousarray(w2[e]).astype(np_dt),
            "b2": np.ascontiguousarray(b2[e]).astype(np.float32),
        })

    res = run_bass_kernel_spmd(nc, in_maps, core_ids=list(range(E)))
    LAST_RESULTS["res"] = res
    LAST_RESULTS["nc"] = nc
    LAST_RESULTS["cap"] = cap

    out = np.zeros((N, C), np.float32)
    for e in range(E):
        ne = len(idxs[e])
        if ne == 0:
            continue
        y = np.asarray(res.results[e]["yT"], np.float32)[:, :ne].T  # [ne, C]
        out[idxs[e]] += gates[e][:, None] * y

    return out.reshape(B, T, C), np.float32(aux_loss)
